# revision 1
# baseline (speedup 1.0000x reference)
"""MixHop layer (powers 0,1,2) Trainium2 Bass kernel.

Problem (per batch b, 8 batches, one NeuronCore each):
    h_p = x_b @ W_p          (x: [F=64, N=2048, T=12], W: [64, 64])
    g_p = adj_b^p @ h_p      (adj: [N, N], diffusion applied p times)
    out_p = leaky_relu(g_p, 0.01)
    out = concat([out_0, out_1, out_2], channel axis) -> [B, 192, N, T]

Design notes:
  - Data-parallel over batch: core b handles batch b.
  - All host-side layout permutations are free (sharding prep); the device
    sees pre-transposed adjacency (adjT, tiled [nb, p, mb, nl]) so the PE's
    lhsT.T @ rhs convention needs no on-chip transposes anywhere.
  - float32r (fp32 with 12-bit mantissa, HW-rounded in the PE) is used for
    all matmuls: 1 cycle/row at free-dim >= 256 vs 4 for plain fp32.
  - Pass A streams adjT once and produces BOTH z1 = adj@h1 (power-1 output)
    and w = adj@h2 (power-2 intermediate) from a packed rhs h12 [m, 1536].
  - Pass B streams adjT again for z2 = adj@w.
  - Outputs are stored in matmul-natural layouts; the host unshard puts
    them back into [B, 192, N, T].
"""

import os
import sys

if "/opt/trn_rl_repo" not in sys.path:
    sys.path.insert(0, "/opt/trn_rl_repo")

import numpy as np

import concourse.bass as bass
import concourse.tile as tile
from concourse import bacc, mybir
from concourse.bass_utils import run_bass_kernel_spmd

F = 64          # input features
O = 64          # output features per power
N = 2048        # nodes
T = 12          # time steps
NB = N // 128   # 16 node blocks
NT = N * T      # 24576
C = O * T       # 768 columns per power, (t, o) ordering

F32 = mybir.dt.float32
F32R = mybir.dt.float32r
LRELU = None  # set at import of mybir below


def build_nc():
    nc = bacc.Bacc("TRN2", target_bir_lowering=False, debug=False, num_devices=8)

    # ---- DRAM I/O ----------------------------------------------------------
    # x2: [(tl, f) = 128, (mb, th, nl) = 12288] where t = 2*th + tl.
    # Stacking two t-planes on the partition axis lets phase 1 run K=128
    # matmuls (full PE rows — keeps the activity monitor / clock gate happy)
    # with a 256-wide packed weight rhs.
    x_d = nc.dram_tensor("x", [128, NT // 2], F32R, kind="ExternalInput").ap()
    # adjT tiled: [nb, p, mb, nl] where adjT[m, n] = adj[n, m], m = mb*128+p,
    # n = nb*128+nl. One [p, (mb nl)] slab per nb is a contiguous 1 MiB read.
    adjt_d = nc.dram_tensor("adjt", [NB, 128, NB, 128], F32R, kind="ExternalInput").ap()
    # wz: [128, 512] = [[wcat, w0, 0], [0, wcat, w0]] block matrix padded to
    # 512 cols (cols 384+ are zero) so the phase-1 matmul (512 cols, 213 ns)
    # fully hides its own 128-col LDWEIGHTS (187 ns).
    wz_d = nc.dram_tensor("wz", [128, 512], F32R, kind="ExternalInput").ap()

    # out0: [n, (t, o)] — same layout as z1/z2
    out0_d = nc.dram_tensor("out0", [N, C], F32, kind="ExternalOutput").ap()
    z1_d = nc.dram_tensor("z1", [N, C], F32, kind="ExternalOutput").ap()       # [n, (t, o)]
    z2_d = nc.dram_tensor("z2", [N, C], F32, kind="ExternalOutput").ap()       # [n, (t, o)]

    lrelu = mybir.ActivationFunctionType.Lrelu

    with tile.TileContext(nc) as tc:
        with (
            tc.tile_pool(name="consts", bufs=1) as consts,
            tc.tile_pool(name="xin", bufs=4) as xin,
            tc.tile_pool(name="h12", bufs=NB) as h12p,
            tc.tile_pool(name="wbuf", bufs=NB) as wbufp,
            tc.tile_pool(name="adjt", bufs=3) as adjp,
            tc.tile_pool(name="zst", bufs=4) as zstp,
            tc.tile_pool(name="p0st", bufs=3) as p0stp,
        ):
            wz_t = consts.tile([128, 512], F32R)
            nc.sync.dma_start(out=wz_t[:], in_=wz_d)

            # ---- Phase 1 + Pass A head (scoped PSUM: 5 small + 3 banks) ----
            # h12 column layout: col = t*128 + z*64 + o  (z=0 -> W1, z=1 -> W2)
            # One K=128 matmul per (mb, th) computes x@W1, x@W2 AND x@W0 for
            # two t-planes (block-diagonal wz rhs). psum cols:
            #   tl*192 + [0:128]   -> (z, o) pair for t = 2*th+tl
            #   tl*192 + [128:192] -> power-0 pre-activation
            # Pass A for nb=0 is interleaved (lagged one mb) to keep PE array
            # duty high from the start (clock-gate governor).
            # preload the first two x tiles ahead of the adjT slab
            x_pre = []
            for mb in range(2):
                x_mb = xin.tile([128, 768], F32R, tag="x", name=f"xpre{mb}")
                nc.sync.dma_start(
                    out=x_mb[:], in_=x_d[:, mb * 768 : (mb + 1) * 768]
                )
                x_pre.append(x_mb)
            slab0 = adjp.tile([128, N], F32R, tag="slab")
            nc.sync.dma_start(
                out=slab0[:], in_=adjt_d[0].rearrange("p a b -> p (a b)")
            )

            # z1/w extraction for a finished pass-A psum tile.
            # psum cols are (t, z, o): z=0 slices -> z1 (leaky), z=1 -> w.
            def drain_passA(pz):
                zt = zstp.tile([128, C], F32, tag="zst")
                nc.scalar.activation(
                    zt[:].rearrange("p (t o) -> p t o", t=T),
                    pz[:].rearrange("p (t z o) -> p t z o", t=T, z=2)[:, :, 0],
                    lrelu,
                    alpha=0.01,
                )
                w_t = wbufp.tile([128, C], F32R, tag="w")
                nc.vector.tensor_copy(
                    w_t[:].rearrange("p (t o) -> p t o", t=T),
                    pz[:]
                    .rearrange("p (t z o) -> p t z o", t=T, z=2)[:, :, 1]
                    .bitcast(F32R),
                )
                return zt, w_t

            h12 = []
            wtiles = []
            with (
                tc.tile_pool(name="ps_a", bufs=1, space="PSUM") as psa,
                tc.tile_pool(name="ps_small", bufs=5, space="PSUM") as pss,
            ):
                pz0 = psa.tile([128, 2 * C], F32, tag="bigA")
                for mb in range(NB):
                    if mb < 2:
                        x_mb = x_pre[mb]
                    else:
                        x_mb = xin.tile([128, 768], F32R, tag="x")
                        nc.sync.dma_start(
                            out=x_mb[:], in_=x_d[:, mb * 768 : (mb + 1) * 768]
                        )
                    h12_t = h12p.tile([128, 2 * C], F32R, tag="h12")
                    h12.append(h12_t)
                    o0 = p0stp.tile([128, C], F32, tag="p0st")
                    for th in range(T // 2):
                        ph = pss.tile([128, 512], F32, tag="small")
                        nc.tensor.matmul(
                            ph[:],
                            x_mb[:, th * 128 : (th + 1) * 128],
                            wz_t[:],
                            start=True,
                            stop=True,
                        )
                        # pass-A head on the PREVIOUS (complete) h12 tile
                        if mb > 0 and th % 2 == 1:
                            hk = th // 2
                            nc.tensor.matmul(
                                pz0[:, hk * 512 : (hk + 1) * 512],
                                slab0[:, (mb - 1) * 128 : mb * 128],
                                h12[mb - 1][:, hk * 512 : (hk + 1) * 512],
                                start=(mb == 1),
                                stop=False,
                            )
                        # h-parts: psum [(tl: step 192) x (z,o): 128] -> h12
                        # contiguous cols [2*th*128, +256)
                        src = ph[:, 0:384].rearrange("p (a b) -> p a b", a=2)[
                            :, :, 0:128
                        ]
                        nc.vector.tensor_copy(
                            h12_t[:, th * 256 : (th + 1) * 256].rearrange(
                                "p (a b) -> p a b", a=2
                            ),
                            src.bitcast(F32R),
                        )
                        # power-0: leaky_relu both tl slices in one ACT
                        nc.scalar.activation(
                            o0[:, 2 * th * O : (2 * th + 2) * O].rearrange(
                                "p (a b) -> p a b", a=2
                            ),
                            ph[:, 0:384].rearrange("p (a b) -> p a b", a=2)[
                                :, :, 128:192
                            ],
                            lrelu,
                            alpha=0.01,
                        )
                    nc.sync.dma_start(
                        out=out0_d[mb * 128 : (mb + 1) * 128, :], in_=o0[:]
                    )
                # flush: last mb's contribution to the head psum tile
                for k in range(3):
                    nc.tensor.matmul(
                        pz0[:, k * 512 : (k + 1) * 512],
                        slab0[:, (NB - 1) * 128 : NB * 128],
                        h12[NB - 1][:, k * 512 : (k + 1) * 512],
                        start=False,
                        stop=(k == 2),
                    )
                zt, w_t = drain_passA(pz0)
                wtiles.append(w_t)
                nc.sync.dma_start(out=z1_d[0:128, :], in_=zt[:])

            psb_cm = tc.tile_pool(name="ps_big", bufs=2, space="PSUM")
            psb = psb_cm.__enter__()
            # ---- Pass A tail: stream adjT for nb = 1..15 -------------------
            for nb in range(1, NB):
                slab = adjp.tile([128, N], F32R, tag="slab")
                nc.sync.dma_start(
                    out=slab[:], in_=adjt_d[nb].rearrange("p a b -> p (a b)")
                )
                pz = psb.tile([128, 2 * C], F32, tag="big")
                for mb in range(NB):
                    lhsT = slab[:, mb * 128 : (mb + 1) * 128]
                    for k in range(3):
                        nc.tensor.matmul(
                            pz[:, k * 512 : (k + 1) * 512],
                            lhsT,
                            h12[mb][:, k * 512 : (k + 1) * 512],
                            start=(mb == 0),
                            stop=(mb == NB - 1),
                        )
                zt, w_t = drain_passA(pz)
                wtiles.append(w_t)
                nc.sync.dma_start(out=z1_d[nb * 128 : (nb + 1) * 128, :], in_=zt[:])

            # ---- Pass B: stream adjT again; z2 = adj@w ---------------------
            for nb in range(NB):
                slab = adjp.tile([128, N], F32R, tag="slab")
                nc.sync.dma_start(
                    out=slab[:], in_=adjt_d[nb].rearrange("p a b -> p (a b)")
                )
                pz = psb.tile([128, 2 * C], F32, tag="big")
                for mb in range(NB):
                    lhsT = slab[:, mb * 128 : (mb + 1) * 128]
                    nc.tensor.matmul(
                        pz[:, 0:512],
                        lhsT,
                        wtiles[mb][:, 0:512],
                        start=(mb == 0),
                        stop=(mb == NB - 1),
                    )
                    nc.tensor.matmul(
                        pz[:, 512:C],
                        lhsT,
                        wtiles[mb][:, 512:C],
                        start=(mb == 0),
                        stop=(mb == NB - 1),
                    )
                zt = zstp.tile([128, C], F32, tag="zst")
                nc.scalar.activation(zt[:], pz[:, 0:C], lrelu, alpha=0.01)
                nc.sync.dma_start(out=z2_d[nb * 128 : (nb + 1) * 128, :], in_=zt[:])
            psb_cm.__exit__(None, None, None)

    nc.finalize()
    return nc


_NC = None
LAST_RESULTS = None  # stashed BassKernelResults for test harnesses


def kernel(x, adj, W0, b0, W1, b1, W2, b2):
    """Full inputs in, full output out. Shards batch b -> core b."""
    global _NC, LAST_RESULTS
    x = np.asarray(x, dtype=np.float32)
    adj = np.asarray(adj, dtype=np.float32)
    W0 = np.asarray(W0, dtype=np.float32)
    W1 = np.asarray(W1, dtype=np.float32)
    W2 = np.asarray(W2, dtype=np.float32)
    b0 = np.asarray(b0, dtype=np.float32)
    b1 = np.asarray(b1, dtype=np.float32)
    b2 = np.asarray(b2, dtype=np.float32)
    B = x.shape[0]
    assert B == 8 and x.shape == (8, F, N, T) and adj.shape == (8, N, N)

    if _NC is None:
        _NC = build_nc()

    # Host-side shard prep (pure layout, free w.r.t. HW time).
    # x: [B, F, N, T] -> [B, (tl, f) = 128, (mb, th, nl)], t = 2*th + tl
    xr = np.ascontiguousarray(
        x.reshape(B, F, NB, 128, T // 2, 2).transpose(0, 5, 1, 2, 4, 3)
    ).reshape(B, 128, NT // 2)
    # adjT tiled: [B, nb, p, mb, nl];  adjT[m, n] = adj[n, m]
    adjt = np.ascontiguousarray(
        adj.transpose(0, 2, 1).reshape(B, NB, 128, NB, 128).transpose(0, 3, 2, 1, 4)
    )
    wcat = np.concatenate([W1, W2], axis=1)  # [64, 128]
    wz = np.zeros((128, 512), dtype=np.float32)
    wz[0:F, 0 : 2 * O] = wcat
    wz[0:F, 2 * O : 3 * O] = W0
    wz[F:128, 3 * O : 5 * O] = wcat
    wz[F:128, 5 * O : 6 * O] = W0

    in_maps = [{"x": xr[b], "adjt": adjt[b], "wz": wz} for b in range(B)]
    nwarm = int(os.environ.get("KERNEL_WARMUP_RUNS", "0"))
    for _ in range(nwarm):
        run_bass_kernel_spmd(_NC, in_maps, core_ids=list(range(8)))
    res = run_bass_kernel_spmd(_NC, in_maps, core_ids=list(range(8)))
    LAST_RESULTS = res

    out = np.empty((B, 3 * O, N, T), dtype=np.float32)
    for b in range(B):
        r = res.results[b]
        # out0: [n, (t, o)] -> [o, n, t]
        out[b, 0:O] = r["out0"].reshape(N, T, O).transpose(2, 0, 1)
        # z1/z2: [n, (t, o)] -> [o, n, t]
        out[b, O : 2 * O] = r["z1"].reshape(N, T, O).transpose(2, 0, 1)
        out[b, 2 * O : 3 * O] = r["z2"].reshape(N, T, O).transpose(2, 0, 1)
    # biases are zero by construction in this problem; nothing to add.
    del b0, b1, b2
    return out



# revision 6
# speedup vs baseline: 1.1075x; 1.1075x over previous
"""MixHop layer (powers 0,1,2) Trainium2 Bass kernel — v2.

Algorithm (per batch b, one NeuronCore each):
    reference: z_p = lrelu(adj^p @ (x @ W_p)),  out = concat_p z_p
    Key identity: adj @ (x @ W) == (adj @ x) @ W  (adj acts on nodes,
    W on features), so diffuse x ONCE per power level instead of
    diffusing each h_p separately:
        y1 = adj @ x          (768 diffusion cols: (t, f))
        y2 = adj @ y1         (768 diffusion cols)
        z0 = lrelu(x @ W0); z1 = lrelu(y1 @ W1); z2 = lrelu(y2 @ W2)
    That is 2x768 diffusion col-passes vs 1536+768 for the naive
    ordering — a 1.5x cut in tensor-engine work.

Implementation notes:
  - All matmul operands are bf16 (1 PE row/cycle, same as fp32r, but
    half the SBUF/DMA bytes); PSUM accumulation stays fp32. adjT is
    SBUF-resident (8.4 MB bf16), read from HBM exactly once and used
    as the moving operand of BOTH diffusion passes.
  - Diffusion runs in "transposed orientation": out y1T[c, n] with
    c=(t,f) on partitions. lhsT = x[m-block, c-block] (natural x
    layout), rhs = adjT[m-block, all n]. The c-layout pairs two t
    planes per 128 partitions (c = t*64+f), so the per-power weight
    matmul is a single [128,128] block-diagonal (W ⊕ W) stationary
    matmul over each y*T c-block — no transposes for z1/z2/z0.
  - The one real transpose (y1T -> y1 natural, needed as pass-2 lhsT)
    uses the DMA XBAR 16x128 transpose path (bf16 SBUF->SBUF): zero
    tensor-engine cost, ~11 us of otherwise-idle DMA time.
  - Matmul outputs are 512-wide (one PSUM bank). Accumulation chains
    run n-chunk-outer / mb-inner so chunk drains (DVE fp32->bf16 copy)
    overlap the next chunk's 16-matmul chain. Pass-1 cb0 runs mb-outer
    instead so its matmuls chase the 16 streaming adjT slab DMAs.
"""

import os
import sys

if "/opt/trn_rl_repo" not in sys.path:
    sys.path.insert(0, "/opt/trn_rl_repo")

import ml_dtypes
import numpy as np

import concourse.bass as bass
import concourse.tile as tile
from concourse import bacc, mybir
from concourse.bass_utils import run_bass_kernel_spmd

F = 64          # input features
O = 64          # output features per power
N = 2048        # nodes
T = 12          # time steps
NB = N // 128   # 16 node blocks
CB = (T // 2)   # 6 c-blocks (t-pair x 64 features/outputs)
C = T * F       # 768 diffusion columns, c = t*64 + f

F32 = mybir.dt.float32
BF16 = mybir.dt.bfloat16


def build_nc():
    nc = bacc.Bacc("TRN2", target_bir_lowering=False, debug=False, num_devices=8)

    # ---- DRAM I/O ----------------------------------------------------------
    # xt: [p, (mb c)] natural x tiled on m, c = t*64+f  (pass-1 lhsT slices)
    xt_d = nc.dram_tensor("xt", [128, NB * C], BF16, kind="ExternalInput").ap()
    # x2: [(tl f), (th n)] for z0, t = 2*th + tl
    x2_d = nc.dram_tensor("x2", [128, CB * N], BF16, kind="ExternalInput").ap()
    # adjt[mb, p, n] = adj[n, mb*128+p] — moving operand of both passes
    adjt_d = nc.dram_tensor("adjt", [NB, 128, N], BF16, kind="ExternalInput").ap()
    # wz: [W1blk | W2blk | W0blk], each [128,128] = W ⊕ W over (tl f)x(tl o)
    wz_d = nc.dram_tensor("wz", [128, 384], BF16, kind="ExternalInput").ap()

    # outputs: [(pair tl o), n] fp32, t = 2*pair + tl
    z0_d = nc.dram_tensor("z0", [C, N], F32, kind="ExternalOutput").ap()
    z1_d = nc.dram_tensor("z1", [C, N], F32, kind="ExternalOutput").ap()
    z2_d = nc.dram_tensor("z2", [C, N], F32, kind="ExternalOutput").ap()

    lrelu = mybir.ActivationFunctionType.Lrelu

    with tile.TileContext(nc) as tc:
        with (
            tc.tile_pool(name="wzp", bufs=1) as wzp,
            tc.tile_pool(name="xtp", bufs=1) as xtp,
            tc.tile_pool(name="x2p", bufs=1) as x2p,
            tc.tile_pool(name="adjp", bufs=1) as adjp,
            tc.tile_pool(name="y1p", bufs=1) as y1p,
            tc.tile_pool(name="y1tp", bufs=2) as y1tp,
            tc.tile_pool(name="y2tp", bufs=2) as y2tp,
            tc.tile_pool(name="zstp", bufs=3) as zstp,
            tc.tile_pool(name="acc", bufs=4, space="PSUM") as accp,
            tc.tile_pool(name="zpp", bufs=2, space="PSUM") as zpp,
        ):
            wz_t = wzp.tile([128, 384], BF16)
            nc.sync.dma_start(out=wz_t[:], in_=wz_d)
            x2_sb = x2p.tile([128, CB * N], BF16)
            nc.sync.dma_start(out=x2_sb[:], in_=x2_d)
            xt_sb = xtp.tile([128, NB * C], BF16)
            nc.sync.dma_start(out=xt_sb[:], in_=xt_d)
            adj_sb = adjp.tile([128, NB * N], BF16)
            for mb in range(NB):
                nc.sync.dma_start(
                    out=adj_sb[:, mb * N : (mb + 1) * N], in_=adjt_d[mb]
                )
            y1_sb = y1p.tile([128, NB * C], BF16)

            def z_block(dst_d, widx, rhs_sb, col0, row0):
                """One [128, N] output block: blockdiag W matmul + lrelu + store."""
                zst = zstp.tile([128, N], F32, tag="zst")
                for s in range(4):
                    zp = zpp.tile([128, 512], F32, tag="zp")
                    nc.tensor.matmul(
                        zp[:],
                        wz_t[:, widx * 128 : (widx + 1) * 128],
                        rhs_sb[:, col0 + s * 512 : col0 + (s + 1) * 512],
                        start=True,
                        stop=True,
                    )
                    nc.scalar.activation(
                        zst[:, s * 512 : (s + 1) * 512], zp[:], lrelu, alpha=0.01
                    )
                nc.sync.dma_start(out=dst_d[row0 : row0 + 128, :], in_=zst[:])

            # ---- z0 = lrelu(x @ W0): rides on x2 while adjT streams in -----
            for th in range(CB):
                z_block(z0_d, 2, x2_sb, th * N, th * 128)

            # ---- pass 1: y1T[c, n] = sum_m x[m, c] adj[n, m] ---------------
            for cb in range(CB):
                y1t = y1tp.tile([128, N], BF16, tag="y1t")
                if cb == 0:
                    # mb-outer: consume adjT slabs in DMA arrival order
                    accs = [
                        accp.tile([128, 512], F32, tag="acc", name=f"acc0_{s}")
                        for s in range(4)
                    ]
                    for mb in range(NB):
                        lhsT = xt_sb[:, mb * C + cb * 128 : mb * C + (cb + 1) * 128]
                        for s in range(4):
                            nc.tensor.matmul(
                                accs[s][:],
                                lhsT,
                                adj_sb[:, mb * N + s * 512 : mb * N + (s + 1) * 512],
                                start=(mb == 0),
                                stop=(mb == NB - 1),
                            )
                    for s in range(4):
                        nc.vector.tensor_copy(
                            y1t[:, s * 512 : (s + 1) * 512], accs[s][:]
                        )
                else:
                    # chunk-outer: staggered drains
                    for s in range(4):
                        acc = accp.tile([128, 512], F32, tag="acc")
                        for mb in range(NB):
                            nc.tensor.matmul(
                                acc[:],
                                xt_sb[:, mb * C + cb * 128 : mb * C + (cb + 1) * 128],
                                adj_sb[:, mb * N + s * 512 : mb * N + (s + 1) * 512],
                                start=(mb == 0),
                                stop=(mb == NB - 1),
                            )
                        nc.vector.tensor_copy(y1t[:, s * 512 : (s + 1) * 512], acc[:])
                z_block(z1_d, 0, y1t, 0, cb * 128)
                # y1T -> y1 natural via DMA XBAR transpose (bf16, SBUF->SBUF)
                for mb in range(NB):
                    nc.sync.dma_start(
                        out=y1_sb[:, mb * C + cb * 128 : mb * C + (cb + 1) * 128],
                        in_=y1t[:, mb * 128 : (mb + 1) * 128],
                        transpose=True,
                    )

            # ---- pass 2: y2T[c, n] = sum_m y1[m, c] adj[n, m] --------------
            for cb in range(CB):
                y2t = y2tp.tile([128, N], BF16, tag="y2t")
                for s in range(4):
                    acc = accp.tile([128, 512], F32, tag="acc")
                    for mb in range(NB):
                        nc.tensor.matmul(
                            acc[:],
                            y1_sb[:, mb * C + cb * 128 : mb * C + (cb + 1) * 128],
                            adj_sb[:, mb * N + s * 512 : mb * N + (s + 1) * 512],
                            start=(mb == 0),
                            stop=(mb == NB - 1),
                        )
                    nc.vector.tensor_copy(y2t[:, s * 512 : (s + 1) * 512], acc[:])
                z_block(z2_d, 1, y2t, 0, cb * 128)

    nc.finalize()
    return nc


_NC = None
LAST_RESULTS = None  # stashed BassKernelResults for test harnesses


def kernel(x, adj, W0, b0, W1, b1, W2, b2):
    """Full inputs in, full output out. Shards batch b -> core b."""
    global _NC, LAST_RESULTS
    x = np.asarray(x, dtype=np.float32)
    adj = np.asarray(adj, dtype=np.float32)
    W0 = np.asarray(W0, dtype=np.float32)
    W1 = np.asarray(W1, dtype=np.float32)
    W2 = np.asarray(W2, dtype=np.float32)
    B = x.shape[0]
    assert B == 8 and x.shape == (8, F, N, T) and adj.shape == (8, N, N)

    if _NC is None:
        _NC = build_nc()

    bf = ml_dtypes.bfloat16
    # xt[b, p, mb*768 + t*64+f] = x[b, f, mb*128+p, t]
    xt = (
        np.ascontiguousarray(
            x.transpose(0, 2, 3, 1).reshape(B, NB, 128, C).transpose(0, 2, 1, 3)
        )
        .reshape(B, 128, NB * C)
        .astype(bf)
    )
    # x2[b, tl*64+f, th*2048+n] = x[b, f, n, 2*th+tl]
    x2 = (
        np.ascontiguousarray(
            x.transpose(0, 3, 1, 2)
            .reshape(B, CB, 2, F, N)
            .transpose(0, 2, 3, 1, 4)
        )
        .reshape(B, 128, CB * N)
        .astype(bf)
    )
    # adjt[b, mb, p, n] = adj[b, n, mb*128+p]
    adjt = (
        np.ascontiguousarray(adj.transpose(0, 2, 1)).reshape(B, NB, 128, N).astype(bf)
    )
    wz = np.zeros((128, 384), dtype=np.float32)
    for tl in range(2):
        r = slice(tl * 64, tl * 64 + 64)
        wz[r, tl * 64 : tl * 64 + 64] = W1
        wz[r, 128 + tl * 64 : 128 + tl * 64 + 64] = W2
        wz[r, 256 + tl * 64 : 256 + tl * 64 + 64] = W0
    wz = wz.astype(bf)

    in_maps = [
        {"xt": xt[b], "x2": x2[b], "adjt": adjt[b], "wz": wz} for b in range(B)
    ]
    nwarm = int(os.environ.get("KERNEL_WARMUP_RUNS", "0"))
    for _ in range(nwarm):
        run_bass_kernel_spmd(_NC, in_maps, core_ids=list(range(8)))
    res = run_bass_kernel_spmd(_NC, in_maps, core_ids=list(range(8)))
    LAST_RESULTS = res

    out = np.empty((B, 3 * O, N, T), dtype=np.float32)
    for b in range(B):
        r = res.results[b]
        for i, key in enumerate(("z0", "z1", "z2")):
            # [(pair tl o), n] -> [o, n, t] with t = 2*pair + tl
            z = r[key].reshape(CB, 2, O, N).transpose(2, 3, 0, 1).reshape(O, N, T)
            out[b, i * O : (i + 1) * O] = z
    del b0, b1, b2
    return out
